# revision 23
# baseline (speedup 1.0000x reference)
"""GQA attention (B=1, T=2048, D=2048, H=32, KVH=8, HD=64) on 8 TRN2 cores.

Head-tensor-parallel: core c owns kv-head c and q-heads 4c..4c+3.
wq/wk/wv column-parallel, wo row-parallel; partials summed on host.

Schedule (per core):
  A: kv projection (4 PSUM banks), paced to the xt DMA stream; PSUM
     evacuated to bf16 via the idle scalar engine so rope runs at the
     DVE 16-bit 2x rate.
  B: q projection in 2 chunk pairs (E/O PSUM double-buffered), rope
     written directly into per-head qT layout (no repack matmuls).
  C: attention per 512-query chunk; head-pair score tiles [128,1024]
     double-buffered against pv accumulators (8 banks total), exp on
     scalar, causal masks on vector, softmax denominator via an
     appended ones-row in the V operand.  Diagonal blocks are
     fine-grained to skip fully-masked query columns.
  D: output projection; final chunk's normalization overlaps the
     first output tiles.
"""
import sys

if "/opt/trn_rl_repo" not in sys.path:
    sys.path.insert(0, "/opt/trn_rl_repo")

import numpy as np
import ml_dtypes

import concourse.bacc as bacc
import concourse.mybir as mybir
import concourse.tile as tile
from concourse.bass_utils import run_bass_kernel_spmd

BF16 = ml_dtypes.bfloat16
T, D, H, KVH, HD = 2048, 2048, 32, 8, 64
NCORES = 8
HPC = H // NCORES            # 4 q heads per core
KT, PT = 16, 128             # k-tiles of 128 over D
NCH = 4                      # t chunks of 512
CH = 512

_cache = {}


def _build_nc():
    if "nc" in _cache:
        return _cache["nc"]
    fp32, bf16 = mybir.dt.float32, mybir.dt.bfloat16
    Exp = mybir.ActivationFunctionType.Exp
    mult = mybir.AluOpType.mult
    nc = bacc.Bacc("TRN2", target_bir_lowering=False, debug=False,
                   num_devices=NCORES)

    xt_d = nc.dram_tensor("xt", [D, T], bf16, kind="ExternalInput")
    wq_d = nc.dram_tensor("wq", [D, HPC * HD], bf16, kind="ExternalInput")
    wkv_d = nc.dram_tensor("wkv", [D, 2 * HD], bf16, kind="ExternalInput")
    wo_d = nc.dram_tensor("wo", [HPC * HD, D], bf16, kind="ExternalInput")
    cs4_d = nc.dram_tensor("cs4", [PT, T], bf16, kind="ExternalInput")
    sn4_d = nc.dram_tensor("sn4", [PT, T], bf16, kind="ExternalInput")
    id_d = nc.dram_tensor("ident", [PT, PT], bf16, kind="ExternalInput")
    mk_d = nc.dram_tensor("masks", [PT, 4, 2 * CH], bf16, kind="ExternalInput")
    out_d = nc.dram_tensor("partial", [T, D], bf16, kind="ExternalOutput")

    with tile.TileContext(nc) as tc:
        with tc.tile_pool(name="const", bufs=1) as const, \
             tc.tile_pool(name="xtp", bufs=KT) as xtp, \
             tc.tile_pool(name="persist", bufs=1) as persist:

            # ---- loads: wkv first, xt stream, wq just before loop1 ----
            wkv_sb = const.tile([PT, KT, 2 * HD], bf16, tag="wkv")
            nc.sync.dma_start(wkv_sb[:], wkv_d.ap().rearrange("(k p) m -> p k m", p=PT))
            xt = []
            for k in range(KT):
                t_ = xtp.tile([PT, T], bf16, tag="xt")
                xt.append(t_)
            for k in range(8):
                nc.sync.dma_start(xt[k][:], xt_d.ap()[k * PT:(k + 1) * PT, :])
            wq_sb = const.tile([PT, KT, HPC * HD], bf16, tag="wq")
            nc.sync.dma_start(wq_sb[:], wq_d.ap().rearrange("(k p) m -> p k m", p=PT))
            for k in range(8, KT):
                nc.sync.dma_start(xt[k][:], xt_d.ap()[k * PT:(k + 1) * PT, :])
            cs4 = const.tile([PT, T], bf16, tag="cs4")
            nc.sync.dma_start(cs4[:], cs4_d.ap())
            sn4 = const.tile([PT, T], bf16, tag="sn4")
            nc.sync.dma_start(sn4[:], sn4_d.ap())
            ident = const.tile([PT, PT], bf16, tag="ident")
            nc.sync.dma_start(ident[:], id_d.ap())
            masks = const.tile([PT, 4, 2 * CH], bf16, tag="masks")
            nc.sync.dma_start(masks[:], mk_d.ap())
            wo_sb = const.tile([PT, 2, D], bf16, tag="wo")
            nc.sync.dma_start(wo_sb[:], wo_d.ap().rearrange("(s p) m -> p s m", p=PT))

            # persistent activations: qtc[j] = [h0|h1|h2|h3] qT for chunk j
            qtc = [persist.tile([64, HPC * CH], bf16, tag=f"qtc{j}", name=f"qtc{j}")
                   for j in range(NCH)]
            kt = persist.tile([64, T], bf16, tag="kt")
            vx = [persist.tile([PT, HD + 1], bf16, tag=f"vx{s}", name=f"vx{s}")
                  for s in range(KT)]
            ot = [persist.tile([PT, T], bf16, tag=f"ot{p}", name=f"ot{p}")
                  for p in range(2)]

            # ---- phase A: kv projection (loop0), PSUM evacuated via scalar ----
            with tc.tile_pool(name="kvcp", bufs=1) as kvcp, \
                 tc.tile_pool(name="tmpa", bufs=2) as tmpa:
                kvb, vt = [], []
                with tc.tile_pool(name="kvp", bufs=1, space="PSUM") as kvp:
                    KV = [kvp.tile([PT, CH], fp32, tag=f"kv{j}", name=f"kv{j}")
                          for j in range(NCH)]
                    for k in range(KT):
                        st, sp = (k == 0), (k == KT - 1)
                        for j in range(NCH):
                            nc.tensor.matmul(KV[j][:], wkv_sb[:, k, :],
                                             xt[k][:, j * CH:(j + 1) * CH],
                                             start=st, stop=sp)
                    for j in range(NCH):
                        ke = kvcp.tile([32, CH], bf16, tag=f"kve{j}", name=f"kve{j}")
                        nc.scalar.copy(ke[:], KV[j][0:32, :])
                        ko = kvcp.tile([32, CH], bf16, tag=f"kvo{j}", name=f"kvo{j}")
                        nc.scalar.copy(ko[:], KV[j][32:64, :])
                        kvb.append((ke, ko))
                        v_ = kvcp.tile([64, CH], bf16, tag=f"vt{j}", name=f"vt{j}")
                        nc.scalar.copy(v_[:], KV[j][64:PT, :])
                        vt.append(v_)
                # k rope on bf16 copies (2x DVE rate)
                for j in range(NCH):
                    jsl = slice(j * CH, (j + 1) * CH)
                    k1 = tmpa.tile([32, CH], bf16, tag="k1")
                    k2 = tmpa.tile([32, CH], bf16, tag="k2")
                    nc.vector.tensor_tensor(k1[:], kvb[j][0][:], cs4[0:32, jsl], mult)
                    nc.vector.tensor_tensor(k2[:], kvb[j][1][:], sn4[0:32, jsl], mult)
                    nc.vector.tensor_sub(kt[0:32, jsl], k1[:], k2[:])
                    k3 = tmpa.tile([32, CH], bf16, tag="k1")
                    k4 = tmpa.tile([32, CH], bf16, tag="k2")
                    nc.vector.tensor_tensor(k3[:], kvb[j][0][:], sn4[0:32, jsl], mult)
                    nc.vector.tensor_tensor(k4[:], kvb[j][1][:], cs4[0:32, jsl], mult)
                    nc.vector.tensor_add(kt[32:64, jsl], k3[:], k4[:])

                # ---- phase B: q projection + rope (direct per-head write) ----
                def q_chunk(j):
                    E = qep.tile([PT, CH], fp32, tag="E", name=f"E{j}")
                    O = qep.tile([PT, CH], fp32, tag="O", name=f"O{j}")
                    jsl = slice(j * CH, (j + 1) * CH)
                    for k in range(KT):
                        st, sp = (k == 0), (k == KT - 1)
                        # re-DMA the slice this chunk reads (identical data):
                        # the matmuls pace to DMA arrival (~60% PE duty), so
                        # the PE banks less heat debt and the hardware
                        # duty-cycle throttle stays disengaged during
                        # attention, which is worth far more than this delay
                        for _ in range(2):
                            nc.sync.dma_start(
                                xt[k][:, jsl],
                                xt_d.ap()[k * PT:(k + 1) * PT, jsl])
                        nc.tensor.matmul(E[:], wq_sb[:, k, 0:PT],
                                         xt[k][:, jsl], start=st, stop=sp)
                        nc.tensor.matmul(O[:], wq_sb[:, k, PT:2 * PT],
                                         xt[k][:, jsl], start=st, stop=sp)
                    return E, O

                def rope_q(j, E, O):
                    jsl = slice(j * CH, (j + 1) * CH)
                    Eb = tmpa.tile([PT, CH], bf16, tag="Eb")
                    Ob = tmpa.tile([PT, CH], bf16, tag="Ob")
                    nc.scalar.copy(Eb[:], E[:])
                    nc.scalar.copy(Ob[:], O[:])
                    t1 = tmpa.tile([PT, CH], bf16, tag="t1")
                    t2 = tmpa.tile([PT, CH], bf16, tag="t2")
                    nc.vector.tensor_tensor(t1[:], Eb[:], cs4[:, jsl], mult)
                    nc.vector.tensor_tensor(t2[:], Ob[:], sn4[:, jsl], mult)
                    for h in range(HPC):
                        nc.vector.tensor_sub(qtc[j][0:32, h * CH:(h + 1) * CH],
                                             t1[32 * h:32 * h + 32, :],
                                             t2[32 * h:32 * h + 32, :])
                    t3 = tmpa.tile([PT, CH], bf16, tag="t1")
                    t4 = tmpa.tile([PT, CH], bf16, tag="t2")
                    nc.vector.tensor_tensor(t3[:], Eb[:], sn4[:, jsl], mult)
                    nc.vector.tensor_tensor(t4[:], Ob[:], cs4[:, jsl], mult)
                    for h in range(HPC):
                        nc.vector.tensor_add(qtc[j][32:64, h * CH:(h + 1) * CH],
                                             t3[32 * h:32 * h + 32, :],
                                             t4[32 * h:32 * h + 32, :])

                with tc.tile_pool(name="qep", bufs=2, space="PSUM") as qep:
                    EO0 = q_chunk(0)
                    EO1 = q_chunk(1)
                    rope_q(0, *EO0)
                    rope_q(1, *EO1)
                    EO2 = q_chunk(2)
                    EO3 = q_chunk(3)
                    with tc.tile_pool(name="vtr", bufs=2, space="PSUM") as vtrp:
                        for j in range(NCH):
                            for u in range(4):
                                s_idx = 4 * j + u
                                vtr = vtrp.tile([PT, 64], bf16, tag="vtr")
                                nc.tensor.transpose(vtr[:], vt[j][:, u * PT:(u + 1) * PT],
                                                    ident[:64, :64])
                                nc.scalar.copy(vx[s_idx][:, 0:HD], vtr[:])
                                nc.vector.memset(vx[s_idx][:, HD:HD + 1], 1.0)
                    rope_q(2, *EO2)
                    rope_q(3, *EO3)

            # ---- phase C: attention (head-pair pipelined, causal fine-grain) ----
            nrm_cm = tc.tile_pool(name="nrm", bufs=2)
            nrm = nrm_cm.__enter__()

            def norm(j, pvs):
                for h in range(HPC):
                    srow = nrm.tile([1, CH], fp32, tag="srow")
                    nc.vector.tensor_copy(srow[:], pvs[h][HD:HD + 1, :])
                    rrow = nrm.tile([1, CH], fp32, tag="rrow")
                    nc.vector.reciprocal_approx_fast(rrow[:], srow[:])
                    bc = nrm.tile([64, CH], fp32, tag="bc")
                    nc.gpsimd.partition_broadcast(bc[:], rrow[:])
                    nc.vector.tensor_tensor(
                        ot[h // 2][64 * (h % 2):64 * (h % 2) + 64,
                                   j * CH:(j + 1) * CH],
                        pvs[h][0:HD, :], bc[:], mult)

            pvs3 = None
            with tc.tile_pool(name="sc", bufs=2, space="PSUM") as scp, \
                 tc.tile_pool(name="pv", bufs=1, space="PSUM") as pvp, \
                 tc.tile_pool(name="ex", bufs=3) as exq:
                for j in range(NCH):
                    pv = [pvp.tile([HD + 1, CH], fp32, tag=f"pv{h}", name=f"pv{h}_{j}")
                          for h in range(HPC)]
                    ilast = 4 * j + 3
                    for i in range(4 * j + 4):
                        ktsl = kt[:, i * PT:(i + 1) * PT]
                        diag = (i // 4 == j)
                        lo = PT * (i % 4) if diag else 0
                        ex_ = []
                        for p in range(2):
                            s = scp.tile([PT, 2 * CH], fp32, tag="sc")
                            for q in range(2):
                                nc.tensor.matmul(
                                    s[:, q * CH + lo:(q + 1) * CH], ktsl,
                                    qtc[j][:, (2 * p + q) * CH + lo:(2 * p + q + 1) * CH],
                                    start=True, stop=True)
                            e = exq.tile([PT, 2 * CH], bf16, tag="ex")
                            nc.scalar.activation(e[:, lo:], s[:, lo:], Exp, scale=0.125)
                            if diag:
                                nc.vector.tensor_tensor(e[:, lo:], e[:, lo:],
                                                        masks[:, i % 4, lo:], mult)
                            ex_.append(e)
                        for p in range(2):
                            for q in range(2):
                                h = 2 * p + q
                                nc.tensor.matmul(pv[h][:, lo:], vx[i],
                                                 ex_[p][:, q * CH + lo:(q + 1) * CH],
                                                 start=(i == 0), stop=(i == ilast),
                                                 skip_group_check=True)
                    # evacuate pv PSUM early (split across vector+scalar) so the
                    # next chunk's accumulation can start without stalling
                    pvs = []
                    for h in range(HPC):
                        t_ = nrm.tile([HD + 1, CH], fp32, tag=f"pvs{h}")
                        if h < 2:
                            nc.vector.tensor_copy(t_[:], pv[h][:])
                        else:
                            nc.scalar.copy(t_[:], pv[h][:])
                        pvs.append(t_)
                    if j < NCH - 1:
                        norm(j, pvs)
                    else:
                        pvs3 = pvs  # normed inside phase D to overlap

            # ---- phase D: output projection, full-row DMA tiles ----
            with tc.tile_pool(name="wp", bufs=4, space="PSUM") as wpp, \
                 tc.tile_pool(name="po", bufs=2) as pop:
                n = 0

                def out_tt(tt, scalar_only=False):
                    nonlocal n
                    pout = pop.tile([PT, D], bf16, tag="po")
                    for dd in range(NCH):
                        wp = wpp.tile([PT, CH], fp32, tag="wp")
                        for s in range(2):
                            nc.tensor.matmul(wp[:], ot[s][:, tt * PT:(tt + 1) * PT],
                                             wo_sb[:, s, dd * CH:(dd + 1) * CH],
                                             start=(s == 0), stop=(s == 1))
                        if scalar_only or n % 2 == 0:
                            nc.scalar.copy(pout[:, dd * CH:(dd + 1) * CH], wp[:])
                        else:
                            nc.vector.tensor_copy(pout[:, dd * CH:(dd + 1) * CH], wp[:])
                        n += 1
                    nc.sync.dma_start(
                        out_d.ap()[tt * PT:(tt + 1) * PT, :], pout[:])

                # norm(j3) first on vector/gpsimd; scalar handles the first
                # tts' PSUM copies so vector is free for it
                norm(NCH - 1, pvs3)
                for tt in range(KT):
                    out_tt(tt, scalar_only=(tt < 2))
            nrm_cm.__exit__(None, None, None)

    nc.compile()
    _cache["nc"] = nc
    return nc


def _host_prep(x, freqs, wq, wk, wv, wo):
    x2d = np.asarray(x, np.float32)[0]                    # [T, D]
    xt = np.ascontiguousarray(x2d.T).astype(BF16)         # [D, T]
    cos = np.cos(np.asarray(freqs, np.float32))           # [T, 32]
    sin = np.sin(np.asarray(freqs, np.float32))
    cs4 = np.ascontiguousarray(np.tile(cos.T, (4, 1)))    # [128, T]
    sn4 = np.ascontiguousarray(np.tile(sin.T, (4, 1)))

    ev, od = np.arange(0, HD, 2), np.arange(1, HD, 2)

    ident = np.eye(PT, dtype=np.float32)

    # masks[sig, r, :] tiled x2 for the head-pair layout
    m1 = np.zeros((PT, 4, CH), np.float32)
    sig = np.arange(PT)[:, None]
    kap = np.arange(CH)[None, :]
    for r in range(4):
        m1[:, r, :] = (kap >= sig + PT * r).astype(np.float32)
    masks = np.ascontiguousarray(np.tile(m1, (1, 1, 2)))  # [128, 4, 1024]

    wq_f = np.asarray(wq, np.float32)
    wk_f = np.asarray(wk, np.float32)
    wv_f = np.asarray(wv, np.float32)
    wo_f = np.asarray(wo, np.float32)

    in_maps = []
    for c in range(NCORES):
        # wq for 4 heads, evens-major-across-heads packing:
        # cols 0:128 = [h0 evens, h1 evens, h2 evens, h3 evens], 128:256 odds
        blocks = [wq_f[:, (c * HPC + h) * HD:(c * HPC + h + 1) * HD] for h in range(HPC)]
        wq_c = np.concatenate([b[:, ev] for b in blocks] + [b[:, od] for b in blocks], axis=1)
        kblk = wk_f[:, c * HD:(c + 1) * HD]
        wkv_c = np.concatenate([kblk[:, ev], kblk[:, od],
                                wv_f[:, c * HD:(c + 1) * HD]], axis=1)
        wo_c = wo_f[c * HPC * HD:(c + 1) * HPC * HD, :]
        in_maps.append({
            "xt": xt,
            "wq": np.ascontiguousarray(wq_c).astype(BF16),
            "wkv": np.ascontiguousarray(wkv_c).astype(BF16),
            "wo": np.ascontiguousarray(wo_c).astype(BF16),
            "cs4": cs4.astype(BF16),
            "sn4": sn4.astype(BF16),
            "ident": ident.astype(BF16),
            "masks": masks.astype(BF16),
        })
    return in_maps


def run(inputs, trace=False, tmpdir=None):
    nc = _build_nc()
    in_maps = _host_prep(**inputs)
    res = run_bass_kernel_spmd(nc, in_maps, list(range(NCORES)),
                               trace=trace, tmpdir=tmpdir)
    acc = np.zeros((T, D), np.float32)
    for c in range(NCORES):
        acc += res.results[c]["partial"].astype(np.float32)
    return acc[None], res


def kernel(**inputs):
    out, _ = run(inputs, trace=False)
    return out


# revision 24
# speedup vs baseline: 1.2963x; 1.2963x over previous
"""GQA attention (B=1, T=2048, D=2048, H=32, KVH=8, HD=64) on 8 TRN2 cores.

Head-tensor-parallel: core c owns kv-head c and q-heads 4c..4c+3.
wq/wk/wv column-parallel, wo row-parallel; partials summed on host.

Schedule (per core):
  A: kv projection (4 PSUM banks), paced to the xt DMA stream; PSUM
     evacuated to bf16 via the idle scalar engine so rope runs at the
     DVE 16-bit 2x rate.
  B: q projection in 2 chunk pairs (E/O PSUM double-buffered), rope
     written directly into per-head qT layout (no repack matmuls).
  C: attention per 512-query chunk; head-pair score tiles [128,1024]
     double-buffered against pv accumulators (8 banks total), exp on
     scalar, causal masks on vector, softmax denominator via an
     appended ones-row in the V operand.  Diagonal blocks are
     fine-grained to skip fully-masked query columns.
  D: output projection; final chunk's normalization overlaps the
     first output tiles.
"""
import sys

if "/opt/trn_rl_repo" not in sys.path:
    sys.path.insert(0, "/opt/trn_rl_repo")

import numpy as np
import ml_dtypes

import concourse.bacc as bacc
import concourse.mybir as mybir
import concourse.tile as tile
from concourse.bass_utils import run_bass_kernel_spmd

BF16 = ml_dtypes.bfloat16
T, D, H, KVH, HD = 2048, 2048, 32, 8, 64
NCORES = 8
HPC = H // NCORES            # 4 q heads per core
KT, PT = 16, 128             # k-tiles of 128 over D
NCH = 4                      # t chunks of 512
CH = 512

_cache = {}


def _build_nc():
    if "nc" in _cache:
        return _cache["nc"]
    fp32, bf16 = mybir.dt.float32, mybir.dt.bfloat16
    Exp = mybir.ActivationFunctionType.Exp
    mult = mybir.AluOpType.mult
    nc = bacc.Bacc("TRN2", target_bir_lowering=False, debug=False,
                   num_devices=NCORES)

    xt_d = nc.dram_tensor("xt", [D, T], bf16, kind="ExternalInput")
    wq_d = nc.dram_tensor("wq", [D, HPC * HD], bf16, kind="ExternalInput")
    wkv_d = nc.dram_tensor("wkv", [D, 2 * HD], bf16, kind="ExternalInput")
    wo_d = nc.dram_tensor("wo", [HPC * HD, D], bf16, kind="ExternalInput")
    cs4_d = nc.dram_tensor("cs4", [PT, T], bf16, kind="ExternalInput")
    sn4_d = nc.dram_tensor("sn4", [PT, T], bf16, kind="ExternalInput")
    id_d = nc.dram_tensor("ident", [PT, PT], bf16, kind="ExternalInput")
    mk_d = nc.dram_tensor("masks", [PT, 4, 2 * CH], bf16, kind="ExternalInput")
    out_d = nc.dram_tensor("partial", [T, D], bf16, kind="ExternalOutput")

    with tile.TileContext(nc) as tc:
        with tc.tile_pool(name="const", bufs=1) as const, \
             tc.tile_pool(name="xtp", bufs=KT) as xtp, \
             tc.tile_pool(name="persist", bufs=1) as persist:

            # ---- loads: wkv first, xt stream, wq just before loop1 ----
            wkv_sb = const.tile([PT, KT, 2 * HD], bf16, tag="wkv")
            nc.sync.dma_start(wkv_sb[:], wkv_d.ap().rearrange("(k p) m -> p k m", p=PT))
            xt = []
            for k in range(KT):
                t_ = xtp.tile([PT, T], bf16, tag="xt")
                xt.append(t_)
            for k in range(8):
                nc.sync.dma_start(xt[k][:], xt_d.ap()[k * PT:(k + 1) * PT, :])
            wq_sb = const.tile([PT, KT, HPC * HD], bf16, tag="wq")
            nc.sync.dma_start(wq_sb[:], wq_d.ap().rearrange("(k p) m -> p k m", p=PT))
            for k in range(8, KT):
                nc.sync.dma_start(xt[k][:], xt_d.ap()[k * PT:(k + 1) * PT, :])
            cs4 = const.tile([PT, T], bf16, tag="cs4")
            nc.sync.dma_start(cs4[:], cs4_d.ap())
            sn4 = const.tile([PT, T], bf16, tag="sn4")
            nc.sync.dma_start(sn4[:], sn4_d.ap())
            ident = const.tile([PT, PT], bf16, tag="ident")
            nc.sync.dma_start(ident[:], id_d.ap())
            masks = const.tile([PT, 4, 2 * CH], bf16, tag="masks")
            nc.sync.dma_start(masks[:], mk_d.ap())
            wo_sb = const.tile([PT, 2, D], bf16, tag="wo")
            nc.sync.dma_start(wo_sb[:], wo_d.ap().rearrange("(s p) m -> p s m", p=PT))

            # persistent activations: qtc[j] = [h0|h1|h2|h3] qT for chunk j
            qtc = [persist.tile([64, HPC * CH], bf16, tag=f"qtc{j}", name=f"qtc{j}")
                   for j in range(NCH)]
            kt = persist.tile([64, T], bf16, tag="kt")
            vx = [persist.tile([PT, HD + 1], bf16, tag=f"vx{s}", name=f"vx{s}")
                  for s in range(KT)]
            ot = [persist.tile([PT, T], bf16, tag=f"ot{p}", name=f"ot{p}")
                  for p in range(2)]

            # ---- phase A: kv projection (loop0), PSUM evacuated via scalar ----
            with tc.tile_pool(name="kvcp", bufs=1) as kvcp, \
                 tc.tile_pool(name="tmpa", bufs=2) as tmpa:
                kvb, vt = [], []
                with tc.tile_pool(name="kvp", bufs=1, space="PSUM") as kvp:
                    KV = [kvp.tile([PT, CH], fp32, tag=f"kv{j}", name=f"kv{j}")
                          for j in range(NCH)]
                    for k in range(KT):
                        st, sp = (k == 0), (k == KT - 1)
                        for j in range(NCH):
                            nc.tensor.matmul(KV[j][:], wkv_sb[:, k, :],
                                             xt[k][:, j * CH:(j + 1) * CH],
                                             start=st, stop=sp)
                    for j in range(NCH):
                        ke = kvcp.tile([32, CH], bf16, tag=f"kve{j}", name=f"kve{j}")
                        nc.scalar.copy(ke[:], KV[j][0:32, :])
                        ko = kvcp.tile([32, CH], bf16, tag=f"kvo{j}", name=f"kvo{j}")
                        nc.scalar.copy(ko[:], KV[j][32:64, :])
                        kvb.append((ke, ko))
                        v_ = kvcp.tile([64, CH], bf16, tag=f"vt{j}", name=f"vt{j}")
                        nc.scalar.copy(v_[:], KV[j][64:PT, :])
                        vt.append(v_)
                # k rope on bf16 copies (2x DVE rate)
                for j in range(NCH):
                    jsl = slice(j * CH, (j + 1) * CH)
                    k1 = tmpa.tile([32, CH], bf16, tag="k1")
                    k2 = tmpa.tile([32, CH], bf16, tag="k2")
                    nc.vector.tensor_tensor(k1[:], kvb[j][0][:], cs4[0:32, jsl], mult)
                    nc.vector.tensor_tensor(k2[:], kvb[j][1][:], sn4[0:32, jsl], mult)
                    nc.vector.tensor_sub(kt[0:32, jsl], k1[:], k2[:])
                    k3 = tmpa.tile([32, CH], bf16, tag="k1")
                    k4 = tmpa.tile([32, CH], bf16, tag="k2")
                    nc.vector.tensor_tensor(k3[:], kvb[j][0][:], sn4[0:32, jsl], mult)
                    nc.vector.tensor_tensor(k4[:], kvb[j][1][:], cs4[0:32, jsl], mult)
                    nc.vector.tensor_add(kt[32:64, jsl], k3[:], k4[:])

                # ---- phase B: q projection + rope (direct per-head write) ----
                def q_chunk(j):
                    E = qep.tile([PT, CH], fp32, tag="E", name=f"E{j}")
                    O = qep.tile([PT, CH], fp32, tag="O", name=f"O{j}")
                    for k in range(KT):
                        st, sp = (k == 0), (k == KT - 1)
                        nc.tensor.matmul(E[:], wq_sb[:, k, 0:PT],
                                         xt[k][:, j * CH:(j + 1) * CH],
                                         start=st, stop=sp)
                        nc.tensor.matmul(O[:], wq_sb[:, k, PT:2 * PT],
                                         xt[k][:, j * CH:(j + 1) * CH],
                                         start=st, stop=sp)
                    return E, O

                def rope_q(j, E, O):
                    jsl = slice(j * CH, (j + 1) * CH)
                    Eb = tmpa.tile([PT, CH], bf16, tag="Eb")
                    Ob = tmpa.tile([PT, CH], bf16, tag="Ob")
                    nc.scalar.copy(Eb[:], E[:])
                    nc.scalar.copy(Ob[:], O[:])
                    t1 = tmpa.tile([PT, CH], bf16, tag="t1")
                    t2 = tmpa.tile([PT, CH], bf16, tag="t2")
                    nc.vector.tensor_tensor(t1[:], Eb[:], cs4[:, jsl], mult)
                    nc.vector.tensor_tensor(t2[:], Ob[:], sn4[:, jsl], mult)
                    for h in range(HPC):
                        nc.vector.tensor_sub(qtc[j][0:32, h * CH:(h + 1) * CH],
                                             t1[32 * h:32 * h + 32, :],
                                             t2[32 * h:32 * h + 32, :])
                    t3 = tmpa.tile([PT, CH], bf16, tag="t1")
                    t4 = tmpa.tile([PT, CH], bf16, tag="t2")
                    nc.vector.tensor_tensor(t3[:], Eb[:], sn4[:, jsl], mult)
                    nc.vector.tensor_tensor(t4[:], Ob[:], cs4[:, jsl], mult)
                    for h in range(HPC):
                        nc.vector.tensor_add(qtc[j][32:64, h * CH:(h + 1) * CH],
                                             t3[32 * h:32 * h + 32, :],
                                             t4[32 * h:32 * h + 32, :])

                with tc.tile_pool(name="qep", bufs=2, space="PSUM") as qep:
                    EO0 = q_chunk(0)
                    EO1 = q_chunk(1)
                    rope_q(0, *EO0)
                    rope_q(1, *EO1)
                    EO2 = q_chunk(2)
                    EO3 = q_chunk(3)
                    with tc.tile_pool(name="vtr", bufs=2, space="PSUM") as vtrp:
                        for j in range(NCH):
                            for u in range(4):
                                s_idx = 4 * j + u
                                vtr = vtrp.tile([PT, 64], bf16, tag="vtr")
                                nc.tensor.transpose(vtr[:], vt[j][:, u * PT:(u + 1) * PT],
                                                    ident[:64, :64])
                                nc.scalar.copy(vx[s_idx][:, 0:HD], vtr[:])
                                nc.vector.memset(vx[s_idx][:, HD:HD + 1], 1.0)
                    rope_q(2, *EO2)
                    rope_q(3, *EO3)

            # ---- phase C: attention (head-pair pipelined, causal fine-grain) ----
            nrm_cm = tc.tile_pool(name="nrm", bufs=2)
            nrm = nrm_cm.__enter__()

            def norm(j, pvs):
                for h in range(HPC):
                    srow = nrm.tile([1, CH], fp32, tag="srow")
                    nc.vector.tensor_copy(srow[:], pvs[h][HD:HD + 1, :])
                    rrow = nrm.tile([1, CH], fp32, tag="rrow")
                    nc.vector.reciprocal_approx_fast(rrow[:], srow[:])
                    bc = nrm.tile([64, CH], fp32, tag="bc")
                    nc.gpsimd.partition_broadcast(bc[:], rrow[:])
                    nc.vector.tensor_tensor(
                        ot[h // 2][64 * (h % 2):64 * (h % 2) + 64,
                                   j * CH:(j + 1) * CH],
                        pvs[h][0:HD, :], bc[:], mult)

            pvs3 = None
            with tc.tile_pool(name="sc", bufs=2, space="PSUM") as scp, \
                 tc.tile_pool(name="pv", bufs=1, space="PSUM") as pvp, \
                 tc.tile_pool(name="ex", bufs=3) as exq:
                for j in range(NCH):
                    pv = [pvp.tile([HD + 1, CH], fp32, tag=f"pv{h}", name=f"pv{h}_{j}")
                          for h in range(HPC)]
                    ilast = 4 * j + 3
                    for i in range(4 * j + 4):
                        ktsl = kt[:, i * PT:(i + 1) * PT]
                        diag = (i // 4 == j)
                        lo = PT * (i % 4) if diag else 0
                        ex_ = []
                        for p in range(2):
                            s = scp.tile([PT, 2 * CH], fp32, tag="sc")
                            for q in range(2):
                                nc.tensor.matmul(
                                    s[:, q * CH + lo:(q + 1) * CH], ktsl,
                                    qtc[j][:, (2 * p + q) * CH + lo:(2 * p + q + 1) * CH],
                                    start=True, stop=True)
                            e = exq.tile([PT, 2 * CH], bf16, tag="ex")
                            nc.scalar.activation(e[:, lo:], s[:, lo:], Exp, scale=0.125)
                            if diag:
                                nc.vector.tensor_tensor(e[:, lo:], e[:, lo:],
                                                        masks[:, i % 4, lo:], mult)
                            ex_.append(e)
                        for p in range(2):
                            for q in range(2):
                                h = 2 * p + q
                                nc.tensor.matmul(pv[h][:, lo:], vx[i],
                                                 ex_[p][:, q * CH + lo:(q + 1) * CH],
                                                 start=(i == 0), stop=(i == ilast),
                                                 skip_group_check=True)
                    # evacuate pv PSUM early (split across vector+scalar) so the
                    # next chunk's accumulation can start without stalling
                    pvs = []
                    for h in range(HPC):
                        t_ = nrm.tile([HD + 1, CH], fp32, tag=f"pvs{h}")
                        if h < 2:
                            nc.vector.tensor_copy(t_[:], pv[h][:])
                        else:
                            nc.scalar.copy(t_[:], pv[h][:])
                        pvs.append(t_)
                    if j < NCH - 1:
                        norm(j, pvs)
                    else:
                        pvs3 = pvs  # normed inside phase D to overlap

            # ---- phase D: output projection, full-row DMA tiles ----
            with tc.tile_pool(name="wp", bufs=4, space="PSUM") as wpp, \
                 tc.tile_pool(name="po", bufs=2) as pop:
                n = 0

                def out_tt(tt, scalar_only=False):
                    nonlocal n
                    pout = pop.tile([PT, D], bf16, tag="po")
                    for dd in range(NCH):
                        wp = wpp.tile([PT, CH], fp32, tag="wp")
                        for s in range(2):
                            nc.tensor.matmul(wp[:], ot[s][:, tt * PT:(tt + 1) * PT],
                                             wo_sb[:, s, dd * CH:(dd + 1) * CH],
                                             start=(s == 0), stop=(s == 1))
                        if scalar_only or n % 2 == 0:
                            nc.scalar.copy(pout[:, dd * CH:(dd + 1) * CH], wp[:])
                        else:
                            nc.vector.tensor_copy(pout[:, dd * CH:(dd + 1) * CH], wp[:])
                        n += 1
                    nc.sync.dma_start(
                        out_d.ap()[tt * PT:(tt + 1) * PT, :], pout[:])

                # norm(j3) first on vector/gpsimd; scalar handles the first
                # tts' PSUM copies so vector is free for it
                norm(NCH - 1, pvs3)
                for tt in range(KT):
                    out_tt(tt, scalar_only=(tt < 2))
            nrm_cm.__exit__(None, None, None)

    nc.compile()
    _cache["nc"] = nc
    return nc


def _host_prep(x, freqs, wq, wk, wv, wo):
    x2d = np.asarray(x, np.float32)[0]                    # [T, D]
    xt = np.ascontiguousarray(x2d.T).astype(BF16)         # [D, T]
    cos = np.cos(np.asarray(freqs, np.float32))           # [T, 32]
    sin = np.sin(np.asarray(freqs, np.float32))
    cs4 = np.ascontiguousarray(np.tile(cos.T, (4, 1)))    # [128, T]
    sn4 = np.ascontiguousarray(np.tile(sin.T, (4, 1)))

    ev, od = np.arange(0, HD, 2), np.arange(1, HD, 2)

    ident = np.eye(PT, dtype=np.float32)

    # masks[sig, r, :] tiled x2 for the head-pair layout
    m1 = np.zeros((PT, 4, CH), np.float32)
    sig = np.arange(PT)[:, None]
    kap = np.arange(CH)[None, :]
    for r in range(4):
        m1[:, r, :] = (kap >= sig + PT * r).astype(np.float32)
    masks = np.ascontiguousarray(np.tile(m1, (1, 1, 2)))  # [128, 4, 1024]

    wq_f = np.asarray(wq, np.float32)
    wk_f = np.asarray(wk, np.float32)
    wv_f = np.asarray(wv, np.float32)
    wo_f = np.asarray(wo, np.float32)

    in_maps = []
    for c in range(NCORES):
        # wq for 4 heads, evens-major-across-heads packing:
        # cols 0:128 = [h0 evens, h1 evens, h2 evens, h3 evens], 128:256 odds
        blocks = [wq_f[:, (c * HPC + h) * HD:(c * HPC + h + 1) * HD] for h in range(HPC)]
        wq_c = np.concatenate([b[:, ev] for b in blocks] + [b[:, od] for b in blocks], axis=1)
        kblk = wk_f[:, c * HD:(c + 1) * HD]
        wkv_c = np.concatenate([kblk[:, ev], kblk[:, od],
                                wv_f[:, c * HD:(c + 1) * HD]], axis=1)
        wo_c = wo_f[c * HPC * HD:(c + 1) * HPC * HD, :]
        in_maps.append({
            "xt": xt,
            "wq": np.ascontiguousarray(wq_c).astype(BF16),
            "wkv": np.ascontiguousarray(wkv_c).astype(BF16),
            "wo": np.ascontiguousarray(wo_c).astype(BF16),
            "cs4": cs4.astype(BF16),
            "sn4": sn4.astype(BF16),
            "ident": ident.astype(BF16),
            "masks": masks.astype(BF16),
        })
    return in_maps


def run(inputs, trace=False, tmpdir=None):
    nc = _build_nc()
    in_maps = _host_prep(**inputs)
    res = run_bass_kernel_spmd(nc, in_maps, list(range(NCORES)),
                               trace=trace, tmpdir=tmpdir)
    acc = np.zeros((T, D), np.float32)
    for c in range(NCORES):
        acc += res.results[c]["partial"].astype(np.float32)
    return acc[None], res


def kernel(**inputs):
    out, _ = run(inputs, trace=False)
    return out


# revision 26
# speedup vs baseline: 1.4342x; 1.1064x over previous
"""GQA attention (B=1, T=2048, D=2048, H=32, KVH=8, HD=64) on 8 TRN2 cores.

Head-tensor-parallel: core c owns kv-head c and q-heads 4c..4c+3.
wq/wk/wv column-parallel, wo row-parallel; partials summed on host.

Schedule (per core):
  A: kv projection (4 PSUM banks), paced to the xt DMA stream; PSUM
     evacuated to bf16 via the idle scalar engine so rope runs at the
     DVE 16-bit 2x rate.
  B: q projection in 2 chunk pairs (E/O PSUM double-buffered), rope
     written directly into per-head qT layout (no repack matmuls).
  C: attention per 512-query chunk; head-pair score tiles [128,1024]
     double-buffered against pv accumulators (8 banks total), exp on
     scalar, causal masks on vector, softmax denominator via an
     appended ones-row in the V operand.  Diagonal blocks are
     fine-grained to skip fully-masked query columns.
  D: output projection; final chunk's normalization overlaps the
     first output tiles.
"""
import sys

if "/opt/trn_rl_repo" not in sys.path:
    sys.path.insert(0, "/opt/trn_rl_repo")

import numpy as np
import ml_dtypes

import concourse.bacc as bacc
import concourse.mybir as mybir
import concourse.tile as tile
from concourse.bass_utils import run_bass_kernel_spmd

BF16 = ml_dtypes.bfloat16
T, D, H, KVH, HD = 2048, 2048, 32, 8, 64
NCORES = 8
HPC = H // NCORES            # 4 q heads per core
KT, PT = 16, 128             # k-tiles of 128 over D
NCH = 4                      # t chunks of 512
CH = 512

_cache = {}


def _build_nc():
    if "nc" in _cache:
        return _cache["nc"]
    fp32, bf16 = mybir.dt.float32, mybir.dt.bfloat16
    Exp = mybir.ActivationFunctionType.Exp
    mult = mybir.AluOpType.mult
    nc = bacc.Bacc("TRN2", target_bir_lowering=False, debug=False,
                   num_devices=NCORES)

    xt_d = nc.dram_tensor("xt", [D, T], bf16, kind="ExternalInput")
    wq_d = nc.dram_tensor("wq", [D, HPC * HD], bf16, kind="ExternalInput")
    wkv_d = nc.dram_tensor("wkv", [D, 2 * HD], bf16, kind="ExternalInput")
    wo_d = nc.dram_tensor("wo", [HPC * HD, D], bf16, kind="ExternalInput")
    cs4_d = nc.dram_tensor("cs4", [PT, T], bf16, kind="ExternalInput")
    sn4_d = nc.dram_tensor("sn4", [PT, T], bf16, kind="ExternalInput")
    id_d = nc.dram_tensor("ident", [PT, PT], bf16, kind="ExternalInput")
    mk_d = nc.dram_tensor("masks", [PT, 4, 2 * CH], bf16, kind="ExternalInput")
    out_d = nc.dram_tensor("partial", [T, D], bf16, kind="ExternalOutput")

    with tile.TileContext(nc) as tc:
        with tc.tile_pool(name="const", bufs=1) as const, \
             tc.tile_pool(name="xtp", bufs=KT) as xtp, \
             tc.tile_pool(name="persist", bufs=1) as persist:

            # ---- loads: wkv first, xt stream, wq just before loop1 ----
            wkv_sb = const.tile([PT, KT, 2 * HD], bf16, tag="wkv")
            nc.sync.dma_start(wkv_sb[:], wkv_d.ap().rearrange("(k p) m -> p k m", p=PT))
            xt = []
            for k in range(KT):
                t_ = xtp.tile([PT, T], bf16, tag="xt")
                xt.append(t_)
            for k in range(8):
                nc.sync.dma_start(xt[k][:], xt_d.ap()[k * PT:(k + 1) * PT, :])
            wq_sb = const.tile([PT, KT, HPC * HD], bf16, tag="wq")
            nc.sync.dma_start(wq_sb[:], wq_d.ap().rearrange("(k p) m -> p k m", p=PT))
            for k in range(8, KT):
                nc.sync.dma_start(xt[k][:], xt_d.ap()[k * PT:(k + 1) * PT, :])
            cs4 = const.tile([PT, T], bf16, tag="cs4")
            nc.sync.dma_start(cs4[:], cs4_d.ap())
            sn4 = const.tile([PT, T], bf16, tag="sn4")
            nc.sync.dma_start(sn4[:], sn4_d.ap())
            ident = const.tile([PT, PT], bf16, tag="ident")
            nc.sync.dma_start(ident[:], id_d.ap())
            masks = const.tile([PT, 4, 2 * CH], bf16, tag="masks")
            nc.sync.dma_start(masks[:], mk_d.ap())
            wo_sb = const.tile([PT, 2, D], bf16, tag="wo")
            nc.sync.dma_start(wo_sb[:], wo_d.ap().rearrange("(s p) m -> p s m", p=PT))

            # persistent activations: qtc[j] = [h0|h1|h2|h3] qT for chunk j
            qtc = [persist.tile([64, HPC * CH], bf16, tag=f"qtc{j}", name=f"qtc{j}")
                   for j in range(NCH)]
            kt = persist.tile([64, T], bf16, tag="kt")
            vx = [persist.tile([PT, HD + 1], bf16, tag=f"vx{s}", name=f"vx{s}")
                  for s in range(KT)]
            ot = [persist.tile([PT, T], bf16, tag=f"ot{p}", name=f"ot{p}")
                  for p in range(2)]

            # ---- phase A: kv projection (loop0) + chunk-0 q E/O folded in so
            # the tensor engine is DMA-paced instead of idling; PSUM
            # evacuated via scalar ----
            with tc.tile_pool(name="kvcp", bufs=1) as kvcp, \
                 tc.tile_pool(name="qep", bufs=2, space="PSUM") as qep, \
                 tc.tile_pool(name="tmpa", bufs=2) as tmpa:
                kvb, vt = [], []
                EO = {0: (qep.tile([PT, CH], fp32, tag="E", name="E0"),
                          qep.tile([PT, CH], fp32, tag="O", name="O0"))}
                Eb = {}
                with tc.tile_pool(name="kvp", bufs=1, space="PSUM") as kvp:
                    KV = [kvp.tile([PT, CH], fp32, tag=f"kv{j}", name=f"kv{j}")
                          for j in range(NCH)]
                    for k in range(KT):
                        st, sp = (k == 0), (k == KT - 1)
                        for j in range(NCH):
                            nc.tensor.matmul(KV[j][:], wkv_sb[:, k, :],
                                             xt[k][:, j * CH:(j + 1) * CH],
                                             start=st, stop=sp)
                        nc.tensor.matmul(EO[0][0][:], wq_sb[:, k, 0:PT],
                                         xt[k][:, 0:CH], start=st, stop=sp)
                        nc.tensor.matmul(EO[0][1][:], wq_sb[:, k, PT:2 * PT],
                                         xt[k][:, 0:CH], start=st, stop=sp)
                    # E0/O0 bf16 evacuation FIRST so the qep rotation for
                    # chunk 2 never waits behind the kv copies
                    e_ = tmpa.tile([PT, CH], bf16, tag="Eb", name="Eb0")
                    o_ = tmpa.tile([PT, CH], bf16, tag="Ob", name="Ob0")
                    nc.scalar.copy(e_[:], EO[0][0][:])
                    nc.scalar.copy(o_[:], EO[0][1][:])
                    Eb[0] = (e_, o_)
                    for j in range(NCH):
                        ke = kvcp.tile([32, CH], bf16, tag=f"kve{j}", name=f"kve{j}")
                        nc.scalar.copy(ke[:], KV[j][0:32, :])
                        ko = kvcp.tile([32, CH], bf16, tag=f"kvo{j}", name=f"kvo{j}")
                        nc.scalar.copy(ko[:], KV[j][32:64, :])
                        kvb.append((ke, ko))
                        v_ = kvcp.tile([64, CH], bf16, tag=f"vt{j}", name=f"vt{j}")
                        nc.scalar.copy(v_[:], KV[j][64:PT, :])
                        vt.append(v_)
                # k rope on bf16 copies (2x DVE rate)
                for j in range(NCH):
                    jsl = slice(j * CH, (j + 1) * CH)
                    k1 = tmpa.tile([32, CH], bf16, tag="k1")
                    k2 = tmpa.tile([32, CH], bf16, tag="k2")
                    nc.vector.tensor_tensor(k1[:], kvb[j][0][:], cs4[0:32, jsl], mult)
                    nc.vector.tensor_tensor(k2[:], kvb[j][1][:], sn4[0:32, jsl], mult)
                    nc.vector.tensor_sub(kt[0:32, jsl], k1[:], k2[:])
                    k3 = tmpa.tile([32, CH], bf16, tag="k1")
                    k4 = tmpa.tile([32, CH], bf16, tag="k2")
                    nc.vector.tensor_tensor(k3[:], kvb[j][0][:], sn4[0:32, jsl], mult)
                    nc.vector.tensor_tensor(k4[:], kvb[j][1][:], cs4[0:32, jsl], mult)
                    nc.vector.tensor_add(kt[32:64, jsl], k3[:], k4[:])

                # ---- phase B: q projection + rope (direct per-head write) ----
                def q_chunk(j):
                    E = qep.tile([PT, CH], fp32, tag="E", name=f"E{j}")
                    O = qep.tile([PT, CH], fp32, tag="O", name=f"O{j}")
                    for k in range(KT):
                        st, sp = (k == 0), (k == KT - 1)
                        nc.tensor.matmul(E[:], wq_sb[:, k, 0:PT],
                                         xt[k][:, j * CH:(j + 1) * CH],
                                         start=st, stop=sp)
                        nc.tensor.matmul(O[:], wq_sb[:, k, PT:2 * PT],
                                         xt[k][:, j * CH:(j + 1) * CH],
                                         start=st, stop=sp)
                    return E, O

                def qcopy(j, E, O):
                    e_ = tmpa.tile([PT, CH], bf16, tag="Eb", name=f"Eb{j}")
                    o_ = tmpa.tile([PT, CH], bf16, tag="Ob", name=f"Ob{j}")
                    nc.scalar.copy(e_[:], E[:])
                    nc.scalar.copy(o_[:], O[:])
                    return e_, o_

                def rope_q_v(j):
                    jsl = slice(j * CH, (j + 1) * CH)
                    eb, ob = Eb[j]
                    t1 = tmpa.tile([PT, CH], bf16, tag="t1")
                    t2 = tmpa.tile([PT, CH], bf16, tag="t2")
                    nc.vector.tensor_tensor(t1[:], eb[:], cs4[:, jsl], mult)
                    nc.vector.tensor_tensor(t2[:], ob[:], sn4[:, jsl], mult)
                    for h in range(HPC):
                        nc.vector.tensor_sub(qtc[j][0:32, h * CH:(h + 1) * CH],
                                             t1[32 * h:32 * h + 32, :],
                                             t2[32 * h:32 * h + 32, :])
                    t3 = tmpa.tile([PT, CH], bf16, tag="t1")
                    t4 = tmpa.tile([PT, CH], bf16, tag="t2")
                    nc.vector.tensor_tensor(t3[:], eb[:], sn4[:, jsl], mult)
                    nc.vector.tensor_tensor(t4[:], ob[:], cs4[:, jsl], mult)
                    for h in range(HPC):
                        nc.vector.tensor_add(qtc[j][32:64, h * CH:(h + 1) * CH],
                                             t3[32 * h:32 * h + 32, :],
                                             t4[32 * h:32 * h + 32, :])

                rope_q_v(0)
                EO[1] = q_chunk(1)
                Eb[1] = qcopy(1, *EO[1])
                rope_q_v(1)
                EO[2] = q_chunk(2)
                Eb[2] = qcopy(2, *EO[2])
                EO[3] = q_chunk(3)
                Eb[3] = qcopy(3, *EO[3])
                with tc.tile_pool(name="vtr", bufs=2, space="PSUM") as vtrp:
                    for j in range(NCH):
                        for u in range(4):
                            s_idx = 4 * j + u
                            vtr = vtrp.tile([PT, 64], bf16, tag="vtr")
                            nc.tensor.transpose(vtr[:], vt[j][:, u * PT:(u + 1) * PT],
                                                ident[:64, :64])
                            nc.scalar.copy(vx[s_idx][:, 0:HD], vtr[:])
                            nc.vector.memset(vx[s_idx][:, HD:HD + 1], 1.0)
                rope_q_v(2)
                rope_q_v(3)

            # ---- phase C: attention (head-pair pipelined, causal fine-grain) ----
            nrm_cm = tc.tile_pool(name="nrm", bufs=2)
            nrm = nrm_cm.__enter__()

            def norm(j, pvs):
                for h in range(HPC):
                    srow = nrm.tile([1, CH], fp32, tag="srow")
                    nc.vector.tensor_copy(srow[:], pvs[h][HD:HD + 1, :])
                    rrow = nrm.tile([1, CH], fp32, tag="rrow")
                    nc.vector.reciprocal_approx_fast(rrow[:], srow[:])
                    bc = nrm.tile([64, CH], fp32, tag="bc")
                    nc.gpsimd.partition_broadcast(bc[:], rrow[:])
                    nc.vector.tensor_tensor(
                        ot[h // 2][64 * (h % 2):64 * (h % 2) + 64,
                                   j * CH:(j + 1) * CH],
                        pvs[h][0:HD, :], bc[:], mult)

            pvs3 = None
            with tc.tile_pool(name="sc", bufs=2, space="PSUM") as scp, \
                 tc.tile_pool(name="pv", bufs=1, space="PSUM") as pvp, \
                 tc.tile_pool(name="ex", bufs=3) as exq:
                for j in range(NCH):
                    pv = [pvp.tile([HD + 1, CH], fp32, tag=f"pv{h}", name=f"pv{h}_{j}")
                          for h in range(HPC)]
                    ilast = 4 * j + 3
                    for i in range(4 * j + 4):
                        ktsl = kt[:, i * PT:(i + 1) * PT]
                        diag = (i // 4 == j)
                        lo = PT * (i % 4) if diag else 0
                        ex_ = []
                        for p in range(2):
                            s = scp.tile([PT, 2 * CH], fp32, tag="sc")
                            for q in range(2):
                                nc.tensor.matmul(
                                    s[:, q * CH + lo:(q + 1) * CH], ktsl,
                                    qtc[j][:, (2 * p + q) * CH + lo:(2 * p + q + 1) * CH],
                                    start=True, stop=True)
                            e = exq.tile([PT, 2 * CH], bf16, tag="ex")
                            nc.scalar.activation(e[:, lo:], s[:, lo:], Exp, scale=0.125)
                            if diag:
                                nc.vector.tensor_tensor(e[:, lo:], e[:, lo:],
                                                        masks[:, i % 4, lo:], mult)
                            ex_.append(e)
                        for p in range(2):
                            for q in range(2):
                                h = 2 * p + q
                                nc.tensor.matmul(pv[h][:, lo:], vx[i],
                                                 ex_[p][:, q * CH + lo:(q + 1) * CH],
                                                 start=(i == 0), stop=(i == ilast),
                                                 skip_group_check=True)
                    # evacuate pv PSUM early (split across vector+scalar) so the
                    # next chunk's accumulation can start without stalling
                    pvs = []
                    for h in range(HPC):
                        t_ = nrm.tile([HD + 1, CH], fp32, tag=f"pvs{h}")
                        if h < 2:
                            nc.vector.tensor_copy(t_[:], pv[h][:])
                        else:
                            nc.scalar.copy(t_[:], pv[h][:])
                        pvs.append(t_)
                    if j < NCH - 1:
                        norm(j, pvs)
                    else:
                        pvs3 = pvs  # normed inside phase D to overlap

            # ---- phase D: output projection, full-row DMA tiles ----
            with tc.tile_pool(name="wp", bufs=4, space="PSUM") as wpp, \
                 tc.tile_pool(name="po", bufs=2) as pop:
                n = 0

                def out_tt(tt, scalar_only=False):
                    nonlocal n
                    pout = pop.tile([PT, D], bf16, tag="po")
                    for dd in range(NCH):
                        wp = wpp.tile([PT, CH], fp32, tag="wp")
                        for s in range(2):
                            nc.tensor.matmul(wp[:], ot[s][:, tt * PT:(tt + 1) * PT],
                                             wo_sb[:, s, dd * CH:(dd + 1) * CH],
                                             start=(s == 0), stop=(s == 1))
                        if scalar_only or n % 2 == 0:
                            nc.scalar.copy(pout[:, dd * CH:(dd + 1) * CH], wp[:])
                        else:
                            nc.vector.tensor_copy(pout[:, dd * CH:(dd + 1) * CH], wp[:])
                        n += 1
                    nc.sync.dma_start(
                        out_d.ap()[tt * PT:(tt + 1) * PT, :], pout[:])

                # norm(j3) first on vector/gpsimd; scalar handles the first
                # tts' PSUM copies so vector is free for it
                norm(NCH - 1, pvs3)
                for tt in range(KT):
                    out_tt(tt, scalar_only=(tt < 2))
            nrm_cm.__exit__(None, None, None)

    nc.compile()
    _cache["nc"] = nc
    return nc


def _host_prep(x, freqs, wq, wk, wv, wo):
    x2d = np.asarray(x, np.float32)[0]                    # [T, D]
    xt = np.ascontiguousarray(x2d.T).astype(BF16)         # [D, T]
    cos = np.cos(np.asarray(freqs, np.float32))           # [T, 32]
    sin = np.sin(np.asarray(freqs, np.float32))
    cs4 = np.ascontiguousarray(np.tile(cos.T, (4, 1)))    # [128, T]
    sn4 = np.ascontiguousarray(np.tile(sin.T, (4, 1)))

    ev, od = np.arange(0, HD, 2), np.arange(1, HD, 2)

    ident = np.eye(PT, dtype=np.float32)

    # masks[sig, r, :] tiled x2 for the head-pair layout
    m1 = np.zeros((PT, 4, CH), np.float32)
    sig = np.arange(PT)[:, None]
    kap = np.arange(CH)[None, :]
    for r in range(4):
        m1[:, r, :] = (kap >= sig + PT * r).astype(np.float32)
    masks = np.ascontiguousarray(np.tile(m1, (1, 1, 2)))  # [128, 4, 1024]

    wq_f = np.asarray(wq, np.float32)
    wk_f = np.asarray(wk, np.float32)
    wv_f = np.asarray(wv, np.float32)
    wo_f = np.asarray(wo, np.float32)

    in_maps = []
    for c in range(NCORES):
        # wq for 4 heads, evens-major-across-heads packing:
        # cols 0:128 = [h0 evens, h1 evens, h2 evens, h3 evens], 128:256 odds
        blocks = [wq_f[:, (c * HPC + h) * HD:(c * HPC + h + 1) * HD] for h in range(HPC)]
        wq_c = np.concatenate([b[:, ev] for b in blocks] + [b[:, od] for b in blocks], axis=1)
        kblk = wk_f[:, c * HD:(c + 1) * HD]
        wkv_c = np.concatenate([kblk[:, ev], kblk[:, od],
                                wv_f[:, c * HD:(c + 1) * HD]], axis=1)
        wo_c = wo_f[c * HPC * HD:(c + 1) * HPC * HD, :]
        in_maps.append({
            "xt": xt,
            "wq": np.ascontiguousarray(wq_c).astype(BF16),
            "wkv": np.ascontiguousarray(wkv_c).astype(BF16),
            "wo": np.ascontiguousarray(wo_c).astype(BF16),
            "cs4": cs4.astype(BF16),
            "sn4": sn4.astype(BF16),
            "ident": ident.astype(BF16),
            "masks": masks.astype(BF16),
        })
    return in_maps


def run(inputs, trace=False, tmpdir=None):
    nc = _build_nc()
    in_maps = _host_prep(**inputs)
    res = run_bass_kernel_spmd(nc, in_maps, list(range(NCORES)),
                               trace=trace, tmpdir=tmpdir)
    acc = np.zeros((T, D), np.float32)
    for c in range(NCORES):
        acc += res.results[c]["partial"].astype(np.float32)
    return acc[None], res


def kernel(**inputs):
    out, _ = run(inputs, trace=False)
    return out
